# revision 34
# baseline (speedup 1.0000x reference)
"""Trainium2 Bass kernel for nn_Mlp_moe (ViT MLP block with MoE-routed cls
tokens), SPMD across 8 NeuronCores.

Sharding:
  - Patch-token MLP (fc1 -> GELU -> fc2): data-parallel over batch
    (8 batches per core). Weights replicated, bf16 compute, fp32 accum.
    fc1 runs h-major (hT = W1 @ xT), fc2 runs token-major with the hT
    tiles as the stationary operand, so y needs no output transpose.
  - Cls/atom MoE path: hidden-dim sharded (each core owns a 384-wide slice
    of every atom's hidden dim, for all 64 batches); per-core partials are
    summed with a ReduceScatter whose output shards line up with each
    core's batch slice. Interleaved between MLP token tiles so the
    collective overlaps the MLP tail.
  - Gate (route logits/softmax/argmax): replicated, fp32, folded into the
    atom path as per-route column scales.
"""

import numpy as np
import ml_dtypes

import bass_rust
import concourse.bass as bass
import concourse.mybir as mybir
import concourse.tile as tile
from concourse.bass_utils import run_bass_kernel_spmd
from concourse.masks import make_identity
from concourse.vector_clock import ScopedClock

F32 = mybir.dt.float32
BF16 = mybir.dt.bfloat16
F8 = mybir.dt.float8e4
AF = mybir.ActivationFunctionType
ALU = mybir.AluOpType
DR = mybir.MatmulPerfMode.DoubleRow

N_CORES = 8
B, T, D, H = 64, 203, 768, 3072
NCLS, NP, NA = 6, 197, 5
BC = B // N_CORES          # batches per core
TOK = BC * T               # 1624 rows of y per core (cls + patch)
NPTOK = BC * NP            # 1576 patch tokens per core
HC = H // N_CORES          # 384 hidden slice per core (cls path)
NTOK_CLS = B * NCLS        # 384 cls tokens globally
KD = D // 128              # 6 k-tiles over D
KH = H // 128              # 24 k-tiles over H
KC = HC // 128             # 3 k-tiles over the per-core hidden slice
FT = 512                   # max MLP token-tile (free dim of fc1 matmuls)
# small first tile starts PE early; remainder folded into the last tile so
# no tile has sub-128 fc1 chains (GELU turnaround can't keep up with those)
TILES = [(0, 512), (512, 512), (1024, 512), (1536, 40)]
NT = len(TILES)
NPTOK_PAD = ((NPTOK + 15) // 16) * 16  # xbar DMA-transpose needs 16-row mult
WARMUP_TRANSPOSES = 30
# --- fp8 DoubleRow config -------------------------------------------------
# F1DR k-chunks (128 wide over D) of fc1 run as fp8e4 DoubleRow matmuls:
# stationary weights carry (hi, lo) e4m3 halves of W1*SW1 in the two k-slots
# (so weights are near-exact), the moving x tile is e4m3-quantized once and
# duplicated into both slots.  Each converted chunk halves that chunk's PE
# rows; error ~= 2.65% * sqrt(F1DR/6) from x quantization only.
F1DR = 2
SW1 = 1024.0  # power-of-2 weight prescale keeping W1 in e4m3 normal range
KB = KD - F1DR             # bf16 k-chunks of fc1
DB = KB * 128              # bf16 D-span of fc1
NPT8 = ((NPTOK + 15) // 16) * 16  # DR moving AP needs 16B-aligned slot step

ATOM = {'vm': 0, 'im': 1, 'cm': 2, 'sc': 3, 'cc': 4}
TASK_PAIRS = [('vm', 'sc'), ('vm', 'cc'), ('im', 'sc'), ('im', 'cc'),
              ('cm', 'sc'), ('cm', 'cc')]
SRC = [[ATOM[l], ATOM[r]] for l, r in TASK_PAIRS]
DST = [[ATOM[r], ATOM[l]] for l, r in TASK_PAIRS]


# ---------------------------------------------------------------------------
# Walrus in this container accepts at most ONE sync-wait per instruction.
# Tile emits multi-wait instructions; split the extras onto preceding
# same-engine wait-nops (engines execute in order, semantics preserved).
# ---------------------------------------------------------------------------

def _patched_drain_and_barrier(self, tick_clock, wait_clock):
    nc = self.nc
    drain_inst = nc.sync.drain()
    wait_clock.add_sem_waits(
        drain_inst.ins, ScopedClock({None: tick_clock.global_clock}))
    si = drain_inst.ins.sync_info
    waits = list(si.on_wait) if si is not None and si.on_wait else []
    if len(waits) > 1:
        drain_inst.ins.sync_info = bass_rust.SyncInfo(
            on_wait=waits[:1], on_update=list(si.on_update or []))
        for w in waits[1:]:
            nop = nc.sync.nop(nofuse=True, hint="drain_wait_split")
            nop.ins.sync_info = bass_rust.SyncInfo(on_wait=[w], on_update=[])
    nc.all_engine_barrier()
    assert self.sems is not None
    popped = nc._tile_sem_poison_stack.pop()
    assert popped is self._sem_poison
    nc.clear_and_free_semaphores(list(self.sems.allocated().values()))
    nc.all_engine_barrier()


tile.TileContext._drain_and_barrier = _patched_drain_and_barrier


def legalize_sync_waits(nc):
    n_split = 0
    for f in nc.m.functions:
        for bb in f.blocks:
            insts = bb.instructions
            new_list = []
            for inst in insts:
                si = inst.sync_info
                waits = list(si.on_wait) if si is not None and si.on_wait else []
                if len(waits) > 1:
                    for w in waits[1:]:
                        eng = nc.engines[inst.engine]
                        nop = eng.nop(nofuse=True, hint="wait_split")
                        cur = nc.cur_bb.bb.instructions
                        assert cur and cur[-1] is nop.ins
                        cur.pop()
                        nop.ins.sync_info = bass_rust.SyncInfo(
                            on_wait=[w], on_update=[])
                        new_list.append(nop.ins)
                        n_split += 1
                    inst.sync_info = bass_rust.SyncInfo(
                        on_wait=waits[:1], on_update=list(si.on_update or []))
                new_list.append(inst)
            if len(new_list) != len(insts):
                insts[:] = new_list
    return n_split


# ---------------------------------------------------------------------------
# Kernel builder
# ---------------------------------------------------------------------------

def build_kernel(debug=False, repeat=1):
    nc = bass.Bass(num_devices=N_CORES)

    x_p = nc.declare_dram_parameter("x_p", [NPTOK_PAD, D], BF16,
                                    isOutput=False)
    x8dp = nc.declare_dram_parameter("x8dp", [F1DR, 128, 2 * NPT8], F8,
                                     isOutput=False) if F1DR else None
    w1drp = nc.declare_dram_parameter("w1drp", [F1DR, 128, 2 * H], F8,
                                      isOutput=False) if F1DR else None
    xclsT = nc.declare_dram_parameter("xclsT", [KD, 128, NTOK_CLS], F32,
                                      isOutput=False)
    w1hp = nc.declare_dram_parameter("w1hp", [KH, 128, DB], BF16,
                                     isOutput=False)
    w2tp = nc.declare_dram_parameter("w2tp", [KH, 128, D], BF16, isOutput=False)
    b1p = nc.declare_dram_parameter("b1p", [128, KH], F32, isOutput=False)
    b1rp = nc.declare_dram_parameter("b1rp", [1, H], BF16, isOutput=False)
    b2rp = nc.declare_dram_parameter("b2rp", [1, D], BF16, isOutput=False)
    b2bcp = nc.declare_dram_parameter("b2bcp", [128, D], F32, isOutput=False)
    b2pp = nc.declare_dram_parameter("b2pp", [128, KD], F32, isOutput=False)
    winp = nc.declare_dram_parameter("winp", [D, NA * HC], BF16, isOutput=False)
    binp = nc.declare_dram_parameter("binp", [128, NA * KC], F32, isOutput=False)
    woutp = nc.declare_dram_parameter("woutp", [KD, 128, NA * HC], BF16,
                                      isOutput=False)
    boutp = nc.declare_dram_parameter("boutp", [1, NA * D], BF16, isOutput=False)
    ghatp = nc.declare_dram_parameter("ghatp", [128, KD * 2 * NCLS], F32,
                                      isOutput=False)
    bbexp = nc.declare_dram_parameter("bbexp", [128, 6], F32, isOutput=False)
    y = nc.declare_dram_parameter("y", [TOK, D], F32, isOutput=True)

    cc_in = nc.dram_tensor("cc_in", [NTOK_CLS, D], BF16)
    cc_out = nc.dram_tensor("cc_out", [NTOK_CLS // N_CORES, D], BF16)

    with tile.TileContext(nc) as tc:
        with tc.tile_pool(name="persist", bufs=1) as pp, \
             tc.tile_pool(name="xT", bufs=2) as xT_p, \
             tc.tile_pool(name="x8", bufs=2) as x8_p, \
             tc.tile_pool(name="ysb", bufs=2) as y_p, \
             tc.tile_pool(name="ps_t", bufs=2, space="PSUM") as ps_t, \
             tc.tile_pool(name="ps_1", bufs=2, space="PSUM") as ps_1, \
             tc.tile_pool(name="ps_2", bufs=2, space="PSUM") as ps_2:

            ident = pp.tile([128, 128], F32, tag="ident", name="ident")
            make_identity(nc, ident)
            ident16 = pp.tile([128, 128], BF16, tag="ident16", name="ident16")
            make_identity(nc, ident16)

            # ---- persistent SBUF tensors -------------------------------
            w1_sb = [pp.tile([128, 4 * DB], BF16, tag=f"w1_{g}",
                             name=f"w1_{g}") for g in range(KH // 4)]
            w1dr_sb = [pp.tile([128, 2 * H], F8, tag=f"w1dr_{k}",
                               name=f"w1dr_{k}") for k in range(F1DR)]
            w2_sb = [pp.tile([128, 4 * D], BF16, tag=f"w2_{g}", name=f"w2_{g}")
                     for g in range(KH // 4)]
            hT = [pp.tile([128, FT], BF16, tag=f"hT_{j}", name=f"hT_{j}")
                  for j in range(KH)]
            xcT32t = pp.tile([128, KD * NTOK_CLS], F32, tag="xcT32",
                             name="xcT32")
            xcT32 = [xcT32t[:, k * NTOK_CLS:(k + 1) * NTOK_CLS]
                     for k in range(KD)]
            xcT16t = pp.tile([128, KD * NTOK_CLS], BF16, tag="xcT16",
                             name="xcT16")
            xcT16 = [xcT16t[:, k * NTOK_CLS:(k + 1) * NTOK_CLS]
                     for k in range(KD)]
            win_sb = [pp.tile([128, NA * HC], BF16, tag=f"win_{k}",
                              name=f"win_{k}") for k in range(KD)]
            SH = [pp.tile([128, 2 * NTOK_CLS], BF16, tag=f"SH_{k}",
                          name=f"SH_{k}") for k in range(KC)]
            b1_sb = pp.tile([128, KH], F32, tag="b1", name="b1")
            b1r_sb = pp.tile([1, H], BF16, tag="b1r", name="b1r")
            b2r_sb = pp.tile([1, D], BF16, tag="b2r", name="b2r")
            b2bc_sb = pp.tile([128, D], F32, tag="b2bc", name="b2bc")
            b2p_sb = pp.tile([128, KD], F32, tag="b2p", name="b2p")
            bin_sb = pp.tile([128, NA * KC], F32, tag="bin", name="bin")
            bout_sb = pp.tile([1, NA * D], BF16, tag="bout", name="bout")
            ghat_sb = pp.tile([128, KD * 12], F32, tag="ghat", name="ghat")
            ones_sb = pp.tile([1, 128], BF16, tag="ones", name="ones")
            w0T_sb = pp.tile([1, NTOK_CLS], F32, tag="w0T", name="w0T")
            w1T_sb_g = pp.tile([1, NTOK_CLS], F32, tag="w1Tg", name="w1Tg")
            w0T16 = pp.tile([1, NTOK_CLS], BF16, tag="w0T16", name="w0T16")
            w1T16 = pp.tile([1, NTOK_CLS], BF16, tag="w1T16", name="w1T16")
            W0b = pp.tile([128, NTOK_CLS], BF16, tag="W0b", name="W0b")
            W1b = pp.tile([128, NTOK_CLS], BF16, tag="W1b", name="W1b")
            zrow_sb = pp.tile([1, 128], BF16, tag="zrow", name="zrow")
            onescol = pp.tile([128, 1], BF16, tag="onescol", name="onescol")
            nc.vector.memset(onescol[:, :], 1.0)
            nc.vector.memset(ones_sb[:, :], 1.0)
            nc.vector.memset(zrow_sb[:, :], 0.0)

            for _rep in range(repeat):
                _emit_iteration(
                    nc, tc, debug,
                    x_p, xclsT, w1hp, w2tp, b1p, b1rp, b2rp, b2bcp, b2pp,
                    winp,
                    binp, woutp, boutp, ghatp, bbexp, y, cc_in, cc_out,
                    pp, xT_p, y_p, ps_t, ps_1, ps_2,
                    ident, ident16, w1_sb, w2_sb, hT, xcT32t, xcT32,
                    xcT16t, xcT16, win_sb, SH, b1_sb, b1r_sb, b2r_sb,
                    b2bc_sb,
                    b2p_sb,
                    bin_sb, bout_sb,
                    ghat_sb, ones_sb, onescol, w0T_sb, w1T_sb_g, w0T16, w1T16,
                    W0b, W1b, zrow_sb, x8dp, w1drp, x8_p, w1dr_sb,
                    load_weights=(_rep == 0))

    legalize_sync_waits(nc)
    return nc


def _emit_iteration(nc, tc, debug,
                    x_p, xclsT, w1hp, w2tp, b1p, b1rp, b2rp, b2bcp, b2pp,
                    winp,
                    binp, woutp, boutp, ghatp, bbexp, y, cc_in, cc_out,
                    pp, xT_p, y_p, ps_t, ps_1, ps_2,
                    ident, ident16, w1_sb, w2_sb, hT, xcT32t, xcT32,
                    xcT16t, xcT16, win_sb, SH, b1_sb, b1r_sb, b2r_sb,
                    b2bc_sb,
                    b2p_sb,
                    bin_sb, bout_sb,
                    ghat_sb, ones_sb, onescol, w0T_sb, w1T_sb_g, w0T16, w1T16,
                    W0b, W1b, zrow_sb, x8dp, w1drp, x8_p, w1dr_sb,
                    load_weights):
    x8_sb = [x8_p.tile([128, 2 * NPT8], F8, tag=f"x8_{k}", name=f"x8_{k}")
             for k in range(F1DR)]
    x8v = [x8_sb[k].rearrange("p (s f) -> p s f", s=2) for k in range(F1DR)]
    w1drv = [w1dr_sb[k].rearrange("p (s h) -> p s h", s=2)
             for k in range(F1DR)]
    def emit_xT(t):
        t0tok, ftl = TILES[t]
        fdma = ((ftl + 15) // 16) * 16
        xTt = xT_p.tile([128, KB * FT], BF16, tag="xT", name=f"xT{t}")
        tv = xTt.rearrange("p (k f) -> p k f", k=KB)
        nc.sync.dma_start(out=tv[:, :, :fdma],
                          in_=x_p[t0tok:t0tok + fdma, F1DR * 128:],
                          transpose=True)
        return xTt, ftl

    # ---- DMA issue order: x tile0, fc1 weights, cls inputs, x tile1,
    # cls weights, fc2 weights, x tail, atom-out weights ----------------
    if load_weights:
        nc.sync.dma_start(out=w1_sb[0][:, 0:DB], in_=w1hp[0, :, :])

    def emit_xT0_split():
        t0tok, ftl = TILES[0]
        xTt = xT_p.tile([128, KB * FT], BF16, tag="xT", name="xT0")
        tv = xTt.rearrange("p (k f) -> p k f", k=KB)
        kmid = KB // 2
        nc.sync.dma_start(out=tv[:, 0:kmid, :],
                          in_=x_p[t0tok:t0tok + ftl,
                                  F1DR * 128:(F1DR + kmid) * 128],
                          transpose=True)
        nc.sync.dma_start(out=tv[:, kmid:KB, :],
                          in_=x_p[t0tok:t0tok + ftl, (F1DR + kmid) * 128:D],
                          transpose=True)
        return xTt, ftl

    xT0 = emit_xT0_split()
    for k in range(F1DR):
        nc.sync.dma_start(out=x8_sb[k][:, :], in_=x8dp[k, :, :])
    if load_weights:
        for k in range(F1DR):
            nc.sync.dma_start(out=w1dr_sb[k][:, :], in_=w1drp[k, :, :])
    if load_weights:
        # keep the PE p-state warm across the DMA-bound startup window:
        # scratch transposes (never read) so the first real chains start
        # at full clock instead of ramping through the slow p-states
        for _ in range(WARMUP_TRANSPOSES):
            pw_ = ps_t.tile([128, 128], BF16, tag="tp", name="warm")
            nc.tensor.transpose(pw_[:, :], ident16[:, :], ident16[:, :])
    if load_weights:
        gv0 = w1_sb[0].rearrange("p (j f) -> p j f", j=4)
        nc.sync.dma_start(out=gv0[:, 1:4, :],
                          in_=w1hp[1:4, :, :].rearrange("j p f -> p j f"))
        nc.sync.dma_start(out=b1_sb[:, :], in_=b1p[:, :])
        nc.sync.dma_start(out=b1r_sb[:, :], in_=b1rp[:, :])
        nc.sync.dma_start(out=b2r_sb[:, :], in_=b2rp[:, :])
        gv1 = w1_sb[1].rearrange("p (j f) -> p j f", j=4)
        nc.sync.dma_start(out=gv1[:, 0:2, :],
                          in_=w1hp[4:6, :, :].rearrange("j p f -> p j f"))
        nc.sync.dma_start(out=gv1[:, 2:4, :],
                          in_=w1hp[6:8, :, :].rearrange("j p f -> p j f"))
        for g in range(2, KH // 4):
            gv = w1_sb[g].rearrange("p (j f) -> p j f", j=4)
            nc.sync.dma_start(
                out=gv[:, :, :],
                in_=w1hp[4 * g:4 * g + 4, :, :].rearrange("j p f -> p j f"))
    if load_weights:
        nc.sync.dma_start(out=ghat_sb[:, :], in_=ghatp[:, :])
        nc.sync.dma_start(out=bin_sb[:, :], in_=binp[:, :])
        nc.sync.dma_start(out=b2p_sb[:, :], in_=b2pp[:, :])
        for g in range(3):
            gv = w2_sb[g].rearrange("p (j f) -> p j f", j=4)
            nc.sync.dma_start(
                out=gv[:, :, :],
                in_=w2tp[4 * g:4 * g + 4, :, :].rearrange("j p f -> p j f"))
    xcv = xcT32t.rearrange("p (k f) -> p k f", k=KD)
    nc.sync.dma_start(out=xcv[:, :, :],
                      in_=xclsT.rearrange("k p f -> p k f"))
    if load_weights:
        nc.sync.dma_start(out=b2bc_sb[:, :], in_=b2bcp[:, :])

    def w1sl(j, k):
        o = (j % 4) * DB + (k - F1DR) * 128
        return w1_sb[j // 4][:, o:o + 128]

    def w2sl(k, half):
        return w2_sb[k // 4][:, (k % 4) * D + half * 384:
                             (k % 4) * D + half * 384 + 384]

    def emit_fc1(t, xTt, ftl):
        t0tok = TILES[t][0]
        for j in range(KH):
            ph = ps_1.tile([128, FT], F32, tag="f1", name=f"f1_{t}_{j}")
            for k in range(F1DR, KD):  # bf16 chunks first: fp8 DMA slack
                nc.tensor.matmul(ph[:, :ftl],
                                 lhsT=w1sl(j, k),
                                 rhs=xTt[:, (k - F1DR) * FT:
                                         (k - F1DR) * FT + ftl],
                                 start=(k == F1DR),
                                 stop=(F1DR == 0 and k == KD - 1))
            for k in range(F1DR):
                nc.tensor.matmul(ph[:, :ftl],
                                 lhsT=w1drv[k][:, :, j * 128:(j + 1) * 128],
                                 rhs=x8v[k][:, :, t0tok:t0tok + ftl],
                                 start=False, stop=(k == F1DR - 1),
                                 perf_mode=DR)
            nc.scalar.activation(hT[j][:, :ftl], ph[:, :ftl], AF.Gelu,
                                 bias=b1_sb[:, j:j + 1], scale=1.0 / SW1)

    def emit_fc2(t, ftl):
        t0tok = TILES[t][0]
        nblk = (ftl + 127) // 128
        for bkl in range(nblk):
            m0 = bkl * 128
            ml = min(128, ftl - m0)
            last = (t == NT - 1 and bkl == nblk - 1)
            ysb = y_p.tile([128, D], F32, tag="ysb", name=f"y_{t}_{bkl}")
            for half in range(2):
                po = ps_2.tile([128, 384], F32, tag="f2",
                               name=f"f2_{t}_{bkl}_{half}")
                for k in range(KH):
                    nc.tensor.matmul(po[:ml, :],
                                     lhsT=hT[k][:, m0:m0 + ml],
                                     rhs=w2sl(k, half),
                                     start=(k == 0), stop=(k == KH - 1))
                cs = slice(half * 384, (half + 1) * 384)
                nc.vector.tensor_tensor(ysb[:ml, cs], po[:ml, :],
                                        b2bc_sb[:ml, cs], ALU.add)
                if last:
                    # final block: store each column-half as soon as it is
                    # ready so the last transfer overlaps the last chains
                    r = t0tok + m0
                    end = r + ml
                    while r < end:
                        b_i, p_i = divmod(r, NP)
                        nxt = min(end, (b_i + 1) * NP)
                        ys = b_i * T + NCLS + p_i
                        nc.scalar.dma_start(
                            out=y[ys:ys + (nxt - r), cs],
                            in_=ysb[r - (t0tok + m0):nxt - (t0tok + m0), cs])
                        r = nxt
            if last:
                continue
            g = t0tok + m0
            end = g + ml
            r = g
            while r < end:
                b_i, p_i = divmod(r, NP)
                nxt = min(end, (b_i + 1) * NP)
                ys = b_i * T + NCLS + p_i
                nc.scalar.dma_start(out=y[ys:ys + (nxt - r), :],
                                    in_=ysb[r - g:nxt - g, :])
                r = nxt

    def emit_fc1_tail(t, xTt, ftl, hTt2):
        t0tok = TILES[t][0]
        # pack 12 j-outputs per PSUM bank; bias lands via rank-1 matmuls so
        # one GELU drains the whole bank (Act per-op overhead would gate PE)
        for grp in range(2):
            ph = ps_1.tile([128, FT], F32, tag="f1", name=f"f1t_{grp}")
            for jj in range(12):
                j = grp * 12 + jj
                nc.tensor.matmul(ph[:, jj * 40:jj * 40 + ftl],
                                 lhsT=b1r_sb[:, j * 128:(j + 1) * 128],
                                 rhs=ones_sb[:, :ftl],
                                 start=(jj == 0), stop=False)
                for k in range(F1DR, KD):
                    nc.tensor.matmul(ph[:, jj * 40:jj * 40 + ftl],
                                     lhsT=w1sl(j, k),
                                     rhs=xTt[:, (k - F1DR) * FT:
                                             (k - F1DR) * FT + ftl],
                                     start=False,
                                     stop=(F1DR == 0 and k == KD - 1))
                for k in range(F1DR):
                    nc.tensor.matmul(ph[:, jj * 40:jj * 40 + ftl],
                                     lhsT=w1drv[k][:, :, j * 128:(j + 1) * 128],
                                     rhs=x8v[k][:, :, t0tok:t0tok + ftl],
                                     start=False, stop=(k == F1DR - 1),
                                     perf_mode=DR)
            nc.scalar.activation(hTt2[grp][:, :12 * 40], ph[:, :12 * 40],
                                 AF.Gelu, scale=1.0 / SW1)

    def emit_fc2_tail(t, ftl, hTt2):
        t0tok = TILES[t][0]
        po = ps_2.tile([128, 384], F32, tag="f2", name="f2t")
        yt = y_p.tile([128, D], BF16, tag="yt16", name="ytail")
        y40 = y_p.tile([128, D], F32, tag="ysb", name="y40")
        for dp in range(KD):
            nc.tensor.matmul(po[:, dp * 40:dp * 40 + ftl],
                             lhsT=b2r_sb[:, dp * 128:(dp + 1) * 128],
                             rhs=ones_sb[:, :ftl],
                             start=(dp == 0), stop=False)
            for k in range(KH):
                nc.tensor.matmul(po[:, dp * 40:dp * 40 + ftl],
                                 lhsT=w2_sb[k // 4][:, (k % 4) * D + dp * 128:
                                                   (k % 4) * D + (dp + 1) * 128],
                                 rhs=hTt2[k // 12][:, (k % 12) * 40:
                                                   (k % 12) * 40 + ftl],
                                 start=False,
                                 stop=(k == KH - 1))
        nc.scalar.activation(yt[:, :KD * 40], po[:, :KD * 40], AF.Identity)
        b_i, p_i = divmod(t0tok, NP)
        ys = b_i * T + NCLS + p_i
        for dp in range(KD):
            ptt = ps_t.tile([128, 128], BF16, tag="tp", name="tp16")
            nc.tensor.transpose(ptt[:ftl, :], yt[:, dp * 40:dp * 40 + ftl],
                                ident16[:, :])
            nc.vector.tensor_copy(y40[:ftl, dp * 128:(dp + 1) * 128],
                                  ptt[:ftl, :])
            if dp == 2:
                nc.sync.dma_start(out=y[ys:ys + ftl, 0:384],
                                  in_=y40[:ftl, 0:384])
        nc.sync.dma_start(out=y[ys:ys + ftl, 384:768], in_=y40[:ftl, 384:768])

    # ================= tile 0: fc1 =====================================
    xTt, ftl = xT0
    emit_fc1(0, xTt, ftl)
    if load_weights:
        for g in range(3, KH // 4):
            gv = w2_sb[g].rearrange("p (j f) -> p j f", j=4)
            nc.sync.dma_start(
                out=gv[:, :, :],
                in_=w2tp[4 * g:4 * g + 4, :, :].rearrange("j p f -> p j f"))
    xT1 = emit_xT(1)

    # ---- cls gate: norms, logits, route weights -----------------------
    with tc.tile_pool(name="cls_tmp", bufs=2) as cls_tmp, \
         tc.tile_pool(name="ps_g", bufs=2, space="PSUM") as ps_g:
        pn = ps_g.tile([128, NTOK_CLS], F32, tag="g", name="pn")
        for k in range(KD):
            nc.vector.tensor_copy(xcT16[k][:, :], xcT32[k][:, :])
            sq = cls_tmp.tile([128, NTOK_CLS], BF16, tag="sq", name=f"sq{k}")
            nc.scalar.activation(sq[:, :], xcT32[k][:, :], AF.Square)
            nc.tensor.matmul(pn[:1, :], lhsT=onescol[:, :], rhs=sq[:, :],
                             start=(k == 0), stop=(k == KD - 1))
        bb_all = cls_tmp.tile([128, 2 * 3], F32, tag="bba", name="bba")
        nc.sync.dma_start(out=bb_all[:, :], in_=bbexp[:, :])
        nrow = cls_tmp.tile([1, NTOK_CLS], F32, tag="nrow", name="nrow")
        nc.scalar.activation(nrow[:1, :], pn[:1, :], AF.Sqrt)
        rnrow = cls_tmp.tile([1, NTOK_CLS], F32, tag="rnrow", name="rnrow")
        nc.vector.reciprocal(rnrow[:1, :], nrow[:1, :])
        for i in range(3):  # 3 tiles of 128 cls tokens, (t,b) order
            ptn_ = ps_g.tile([128, NTOK_CLS], F32, tag="g", name="ptn")
            nc.tensor.transpose(ptn_[:, 0:1], rnrow[:1, i * 128:(i + 1) * 128],
                                ident[:1, :1])
            rn = cls_tmp.tile([128, 1], F32, tag="rn", name="rn")
            nc.vector.tensor_copy(rn[:, :], ptn_[:, 0:1])

            pgt = ps_g.tile([128, NTOK_CLS], F32, tag="g", name="pg")
            pg = pgt[:, 0:12]
            for k in range(KD):
                nc.tensor.matmul(
                    pg[:, :],
                    lhsT=xcT32[k][:, i * 128:(i + 1) * 128],
                    rhs=ghat_sb[:, k * 12:(k + 1) * 12],
                    start=(k == 0), stop=(k == KD - 1))
            lg = cls_tmp.tile([128, 12], F32, tag="lg", name="lg")
            nc.vector.tensor_scalar_mul(lg[:, :], pg[:, :], rn[:, :])

            bb_sb = bb_all[:, 2 * i:2 * i + 2]
            d01 = cls_tmp.tile([128, 2], F32, tag="d01", name="d01")
            t0, t1 = 2 * i, 2 * i + 1
            nc.vector.tensor_tensor(d01[0:64, :], lg[0:64, 2 * t0:2 * t0 + 2],
                                    bb_sb[0:64, :], ALU.add)
            nc.vector.tensor_tensor(d01[64:128, :],
                                    lg[64:128, 2 * t1:2 * t1 + 2],
                                    bb_sb[64:128, :], ALU.add)
            diff = cls_tmp.tile([128, 1], F32, tag="diff", name="diff")
            nc.vector.tensor_tensor(diff[:, :], d01[:, 0:1], d01[:, 1:2],
                                    ALU.subtract)
            ad = cls_tmp.tile([128, 1], F32, tag="ad", name="ad")
            nc.scalar.activation(ad[:, :], diff[:, :], AF.Abs)
            pmax = cls_tmp.tile([128, 1], F32, tag="pmax", name="pmax")
            nc.scalar.activation(pmax[:, :], ad[:, :], AF.Sigmoid)
            m0 = cls_tmp.tile([128, 1], F32, tag="m0", name="m0")
            nc.vector.tensor_scalar(m0[:, :], diff[:, :], 0.0, None, ALU.is_ge)
            w0 = cls_tmp.tile([128, 1], F32, tag="w0", name="w0")
            nc.vector.tensor_tensor(w0[:, :], m0[:, :], pmax[:, :], ALU.mult)
            w1g = cls_tmp.tile([128, 1], F32, tag="w1g", name="w1g")
            nc.vector.tensor_tensor(w1g[:, :], pmax[:, :], w0[:, :],
                                    ALU.subtract)
            ptw_ = ps_g.tile([128, NTOK_CLS], F32, tag="g", name="tpf")
            ptw = ptw_[:, 0:128]
            nc.tensor.transpose(ptw[:1, :], w0[:, 0:1], ident[:, :])
            nc.vector.tensor_copy(w0T_sb[:, i * 128:(i + 1) * 128], ptw[:1, :])
            ptw2_ = ps_g.tile([128, NTOK_CLS], F32, tag="g", name="tpf")
            ptw2 = ptw2_[:, 0:128]
            nc.tensor.transpose(ptw2[:1, :], w1g[:, 0:1], ident[:, :])
            nc.vector.tensor_copy(w1T_sb_g[:, i * 128:(i + 1) * 128],
                                  ptw2[:1, :])

        nc.vector.tensor_copy(w0T16[:, :], w0T_sb[:, :])
        nc.vector.tensor_copy(w1T16[:, :], w1T_sb_g[:, :])

        # broadcast w0/w1 across partitions, in SH column order.
        pw = ps_g.tile([128, NTOK_CLS], F32, tag="g", name="pw")
        ev = w0T16.rearrange("p (t b) -> p t b", b=64)
        nc.tensor.matmul(pw[:, 0:192], lhsT=ones_sb[:, :],
                         rhs=ev[:, 0:6:2, :], start=True, stop=True)
        nc.tensor.matmul(pw[:, 192:384], lhsT=ones_sb[:, :],
                         rhs=ev[:, 1:6:2, :], start=True, stop=True)
        nc.vector.tensor_copy(W0b[:, :], pw[:, :])
        pw2 = ps_g.tile([128, NTOK_CLS], F32, tag="g", name="pw")
        nc.tensor.matmul(pw2[:, :], lhsT=ones_sb[:, :],
                         rhs=w1T16[:, :], start=True, stop=True)
        nc.vector.tensor_copy(W1b[:, :], pw2[:, :])

    # ================= tile 0 fc2, tile 1 ==============================
    emit_fc2(0, ftl)
    xTt, ftl = xT1
    emit_fc1(1, xTt, ftl)
    xT2 = emit_xT(2)
    for k in range(KD):
        nc.sync.dma_start(out=win_sb[k][:, :],
                          in_=winp[k * 128:(k + 1) * 128, :])
    xT3 = emit_xT(3)
    if load_weights:
        nc.sync.dma_start(out=bout_sb[:, :], in_=boutp[:, :])

    # ---- cls stage-1: hid in SH layout --------------------------------
    # SH col layout: [dst3: t0,t2,t4 (192)][dst4: t1,t3,t5 (192)]
    #                [dst0: t0,t1 (128)][dst1: t2,t3][dst2: t4,t5]
    with tc.tile_pool(name="ps_s1", bufs=2, space="PSUM") as ps_s1:
        xvt = xcT16t.rearrange("p (k t b) -> p k t b", k=KD, b=64)
        xv = [xvt[:, k] for k in range(KD)]
        for a in range(NA):
            # pack two m-chains per PSUM tile: doubles the GELU drain
            # deadline so the Act round-trip never gates the ring
            for mp in ((0, 1), (2,)):
                na = 128 if a <= 2 else 192
                ph = ps_s1.tile([128, 384], F32, tag="s1",
                                name=f"s1_{a}_{mp[0]}")
                for idx, m in enumerate(mp):
                    for k in range(KD):
                        if a <= 2:
                            rhs = xcT16[k][:, a * 128:(a + 1) * 128]
                        else:
                            rhs = xv[k][:, (a - 3):NCLS:2, :]
                        nc.tensor.matmul(
                            ph[:, idx * 192:idx * 192 + na],
                            lhsT=win_sb[k][:, a * HC + m * 128:
                                           a * HC + (m + 1) * 128],
                            rhs=rhs,
                            start=(idx == 0 and k == 0),
                            stop=(k == KD - 1))
                for idx, m in enumerate(mp):
                    o = idx * 192
                    bias = bin_sb[:, a * KC + m: a * KC + m + 1]
                    if a <= 2:
                        # route-0 hid: task 2a -> dst3, 2a+1 -> dst4 group
                        nc.scalar.activation(
                            SH[m][:, a * 64:(a + 1) * 64],
                            ph[:, o:o + 64], AF.Gelu, bias=bias)
                        nc.scalar.activation(
                            SH[m][:, 192 + a * 64:192 + (a + 1) * 64],
                            ph[:, o + 64:o + 128], AF.Gelu, bias=bias)
                    else:
                        # route-1 hid: task t at 384 + (t//2)*128 + (t%2)*64
                        off = 64 * (a - 3)
                        for g in range(3):
                            nc.scalar.activation(
                                SH[m][:, 384 + g * 128 + off:
                                      384 + g * 128 + off + 64],
                                ph[:, o + g * 64:o + (g + 1) * 64],
                                AF.Gelu, bias=bias)
        for m in range(KC):
            nc.vector.tensor_tensor(SH[m][:, 0:384], SH[m][:, 0:384],
                                    W0b[:, :], ALU.mult)
            nc.vector.tensor_tensor(SH[m][:, 384:768], SH[m][:, 384:768],
                                    W1b[:, :], ALU.mult)

    # atom-out weights overwrite the atom-in tiles (sequential lifetimes)
    for k in range(KD):
        nc.sync.dma_start(out=win_sb[k][:, :], in_=woutp[k, :, :])

    def wout_ap(j, dp):
        flat = j * D + dp * 128
        return win_sb[flat // (NA * HC)][:, flat % (NA * HC):
                                         flat % (NA * HC) + 128]

    # ================= tile 1 fc2, tile 2 ==============================
    emit_fc2(1, ftl)
    xTt, ftl = xT2
    emit_fc1(2, xTt, ftl)

    # ---- cls stage-2: partial outputs, transpose, RS ------------------
    with tc.tile_pool(name="ps_s2", bufs=2, space="PSUM") as ps_s2, \
         tc.tile_pool(name="fin", bufs=2) as fin:
        pt_sb = [fin.tile([128, NTOK_CLS], BF16, tag=f"pt_{dp}",
                          name=f"pt_{dp}") for dp in range(KD)]
        shr = [SH[k].rearrange("p (q b) -> p q b", b=64) for k in range(KC)]
        w0r = w0T16.rearrange("p (t b) -> p t b", b=64)
        for dp in range(KD):
            pout = ps_s2.tile([128, NTOK_CLS], F32, tag="po", name=f"po_{dp}")
            nc.tensor.matmul(pout[:, :], lhsT=zrow_sb[:, :],
                             rhs=W0b[:1, :], start=True, stop=False)
            po = pout.rearrange("p (t b) -> p t b", b=64)
            for ai, a in enumerate((3, 4)):
                out_ap = po[:, ai:NCLS:2, :]
                for k in range(KC):
                    nc.tensor.matmul(
                        out_ap,
                        lhsT=wout_ap(a * KC + k, dp),
                        rhs=shr[k][:, 3 * ai:3 * (ai + 1), :],
                        start=False, stop=False)
            for a in range(3):
                out_ap = pout[:, a * 128:(a + 1) * 128]
                for k in range(KC):
                    nc.tensor.matmul(
                        out_ap,
                        lhsT=wout_ap(a * KC + k, dp),
                        rhs=SH[k][:, 384 + a * 128:384 + (a + 1) * 128],
                        start=False, stop=False)
            for ai, a in enumerate((3, 4)):
                nc.tensor.matmul(
                    po[:, ai:NCLS:2, :],
                    lhsT=bout_sb[:, a * D + dp * 128:a * D + (dp + 1) * 128],
                    rhs=w0r[:, ai:NCLS:2, :],
                    start=False, stop=False)
            for a in range(3):
                nc.tensor.matmul(
                    pout[:, a * 128:(a + 1) * 128],
                    lhsT=bout_sb[:, a * D + dp * 128:a * D + (dp + 1) * 128],
                    rhs=w1T16[:, a * 128:(a + 1) * 128],
                    start=False, stop=(a == 2))
            nc.vector.tensor_copy(pt_sb[dp][:, :], pout[:, :])

        for i in range(3):  # token blocks of 128 (t,b order)
            o3 = fin.tile([128, D], BF16, tag="o3", name=f"o3{i}")
            for dp in range(KD):
                ptt = ps_t.tile([128, 128], BF16, tag="tp", name="tp16")
                nc.tensor.transpose(ptt[:, :],
                                    pt_sb[dp][:, i * 128:(i + 1) * 128],
                                    ident16[:, :])
                nc.vector.tensor_copy(o3[:, dp * 128:(dp + 1) * 128],
                                      ptt[:, :])
            for half in range(2):
                t = 2 * i + half
                cc_view = cc_in.rearrange("(b t) d -> b t d", t=NCLS)
                nc.scalar.dma_start(
                    out=cc_view[:, t, :],
                    in_=o3[half * 64:(half + 1) * 64, :])

        nc.gpsimd.collective_compute(
            "ReduceScatter", ALU.add,
            replica_groups=[list(range(N_CORES))],
            ins=[cc_in[:, :]], outs=[cc_out[:, :]])

    # ================= tile 2 fc2, tile 3 (tail) =======================
    emit_fc2(2, ftl)
    xTt, ftl = xT3
    hTt2 = [pp.tile([128, 12 * 40], BF16, tag=f"hTt_{g}", name=f"hTt_{g}")
            for g in range(2)]
    emit_fc1_tail(3, xTt, ftl, hTt2)
    emit_fc2_tail(3, ftl, hTt2)

    # ---- cls output: convert RS result to f32, store ------------------
    with tc.tile_pool(name="cout", bufs=1) as cout_p:
        cco16 = cout_p.tile([NTOK_CLS // N_CORES, D], BF16, tag="cco16",
                            name="cco16")
        nc.scalar.dma_start(out=cco16[:, :], in_=cc_out[:, :])
        cco32 = cout_p.tile([NTOK_CLS // N_CORES, D], F32, tag="cco32",
                            name="cco32")
        nc.vector.tensor_copy(cco32[:, :], cco16[:, :])
        y_view = y.rearrange("(b t) d -> b t d", t=T)
        nc.scalar.dma_start(out=y_view[:, 0:NCLS, :], in_=cco32[:, :])


# ---------------------------------------------------------------------------
# Host side
# ---------------------------------------------------------------------------

_CACHE = {}


def _prep_inputs(x, fc1_w, fc1_b, fc2_w, fc2_b, gate_pair, atom_in_w, atom_in_b,
                 atom_out_w, atom_out_b, balance_bias):
    bf = ml_dtypes.bfloat16
    x = np.asarray(x, np.float32)
    w1T = np.asarray(fc1_w, np.float32).T  # [D, H]
    f8 = ml_dtypes.float8_e4m3
    w1Ts = w1T * SW1  # prescaled so fp8-DR and bf16 k-chunks share one scale
    common = {
        # fc1 bf16 weights (k-chunks F1DR..KD), h-major tiles:
        # w1hp[j][dk, kb*128+hj] = W1T[(F1DR+kb)*128+dk, j*128+hj] * SW1
        "w1hp": np.ascontiguousarray(
            w1Ts.reshape(KD, 128, KH, 128)[F1DR:].transpose(2, 1, 0, 3)
            .reshape(KH, 128, DB)).astype(bf),
        # fc2 weights as rhs tiles: w2tp[k] = fc2_w.T[k*128:(k+1)*128, :]
        "w2tp": np.ascontiguousarray(
            np.asarray(fc2_w, np.float32).T.reshape(KH, 128, D)).astype(bf),
        "b1p": np.ascontiguousarray(
            np.asarray(fc1_b, np.float32).reshape(KH, 128).T),
        "b1rp": (np.asarray(fc1_b, np.float32) * SW1).reshape(1, H).astype(bf),
        "b2rp": np.asarray(fc2_b, np.float32).reshape(1, D).astype(bf),
        "b2bcp": np.ascontiguousarray(
            np.broadcast_to(np.asarray(fc2_b, np.float32)[None, :], (128, D))),
        "b2pp": np.ascontiguousarray(
            np.asarray(fc2_b, np.float32).reshape(KD, 128).T),
        "boutp": (np.asarray(atom_out_b, np.float32) / N_CORES)
            .reshape(1, NA * D).astype(bf),
        "bbexp": np.ascontiguousarray(
            np.repeat(np.asarray(balance_bias, np.float32), B, axis=0)
            .reshape(3, 128, 2).transpose(1, 0, 2).reshape(128, 6)),
    }
    if F1DR:
        wch = np.ascontiguousarray(w1Ts.reshape(KD, 128, H)[:F1DR])
        hi = wch.astype(f8)
        lo = (wch - hi.astype(np.float32)).astype(f8)
        common["w1drp"] = np.ascontiguousarray(
            np.stack([hi, lo], axis=2).reshape(F1DR, 128, 2 * H))
    g = np.asarray(gate_pair, np.float32)
    gn = g / np.clip(np.linalg.norm(g, axis=-1, keepdims=True), 1e-12, None)
    ghatT = gn.reshape(2 * NCLS, D).T  # [D, 12]
    common["ghatp"] = np.ascontiguousarray(
        ghatT.reshape(KD, 128, 2 * NCLS).transpose(1, 0, 2)
        .reshape(128, KD * 2 * NCLS))
    # cls tokens for all batches in (t, b) order
    xc = np.asarray(x[:, :NCLS, :], np.float32)  # [B, 6, D]
    common["xclsT"] = np.ascontiguousarray(
        xc.transpose(1, 0, 2).reshape(NTOK_CLS, D).T.reshape(KD, 128,
                                                             NTOK_CLS))

    aiw = np.asarray(atom_in_w, np.float32)   # [5, H, D]
    aib = np.asarray(atom_in_b, np.float32)   # [5, H]
    aow = np.asarray(atom_out_w, np.float32)  # [5, D, H]

    in_maps = []
    for c in range(N_CORES):
        hs = slice(c * HC, (c + 1) * HC)
        m = dict(common)
        xp = np.zeros((NPTOK_PAD, D), np.float32)
        xp[:NPTOK] = x[c * BC:(c + 1) * BC, NCLS:, :].reshape(NPTOK, D)
        m["x_p"] = xp.astype(bf)
        if F1DR:
            x8c = np.zeros((F1DR, 128, 2 * NPT8), f8)
            for k in range(F1DR):
                q = np.ascontiguousarray(
                    xp[:NPTOK, k * 128:(k + 1) * 128].T).astype(f8)
                x8c[k, :, :NPTOK] = q
                x8c[k, :, NPT8:NPT8 + NPTOK] = q
            m["x8dp"] = x8c
        m["winp"] = np.ascontiguousarray(
            aiw[:, hs, :].transpose(2, 0, 1).reshape(D, NA * HC)).astype(bf)
        m["binp"] = np.ascontiguousarray(
            aib[:, hs].reshape(NA, KC, 128).transpose(2, 0, 1)
            .reshape(128, NA * KC))
        blocks = (aow[:, :, hs].transpose(0, 2, 1)
                  .reshape(NA, KC, 128, D).reshape(NA * KC, 128, D))
        flatcols = np.concatenate(list(blocks), axis=1)  # [128, 11520]
        m["woutp"] = np.ascontiguousarray(
            flatcols.reshape(128, KD, NA * HC).swapaxes(0, 1)
        ).astype(bf)
        in_maps.append(m)
    return in_maps


def _get_nc():
    if "nc" not in _CACHE:
        _CACHE["nc"] = build_kernel()
    return _CACHE["nc"]


def kernel(**inputs) -> np.ndarray:
    nc = _get_nc()
    in_maps = _prep_inputs(**inputs)
    res = run_bass_kernel_spmd(nc, in_maps, core_ids=list(range(N_CORES)))
    out = np.empty((B, T, D), np.float32)
    for c in range(N_CORES):
        out[c * BC:(c + 1) * BC] = res.results[c]["y"].reshape(BC, T, D)
    return out


if __name__ == "__main__":
    nc = build_kernel()
    n = sum(len(bb.instructions) for f in nc.m.functions for bb in f.blocks)
    print("instructions:", n)



# revision 41
# speedup vs baseline: 1.3705x; 1.3705x over previous
"""Trainium2 Bass kernel for nn_Mlp_moe (ViT MLP block with MoE-routed cls
tokens), SPMD across 8 NeuronCores.

Sharding:
  - Patch-token MLP (fc1 -> GELU -> fc2): data-parallel over batch
    (8 batches per core). Weights replicated, bf16 compute, fp32 accum.
    fc1 runs h-major (hT = W1 @ xT), fc2 runs token-major with the hT
    tiles as the stationary operand, so y needs no output transpose.
  - Cls/atom MoE path: hidden-dim sharded (each core owns a 384-wide slice
    of every atom's hidden dim, for all 64 batches); per-core partials are
    summed with a ReduceScatter whose output shards line up with each
    core's batch slice. Interleaved between MLP token tiles so the
    collective overlaps the MLP tail.
  - Gate (route logits/softmax/argmax): replicated, fp32, folded into the
    atom path as per-route column scales.
"""

import numpy as np
import ml_dtypes

import bass_rust
import concourse.bass as bass
import concourse.mybir as mybir
import concourse.tile as tile
from concourse.bass_utils import run_bass_kernel_spmd
from concourse.masks import make_identity
from concourse.vector_clock import ScopedClock

F32 = mybir.dt.float32
BF16 = mybir.dt.bfloat16
AF = mybir.ActivationFunctionType
ALU = mybir.AluOpType

N_CORES = 8
B, T, D, H = 64, 203, 768, 3072
NCLS, NP, NA = 6, 197, 5
BC = B // N_CORES          # batches per core
TOK = BC * T               # 1624 rows of y per core (cls + patch)
NPTOK = BC * NP            # 1576 patch tokens per core
HC = H // N_CORES          # 384 hidden slice per core (cls path)
NTOK_CLS = B * NCLS        # 384 cls tokens globally
KD = D // 128              # 6 k-tiles over D
KH = H // 128              # 24 k-tiles over H
KC = HC // 128             # 3 k-tiles over the per-core hidden slice
FT = 512                   # max MLP token-tile (free dim of fc1 matmuls)
# small first tile starts PE early; remainder folded into the last tile so
# no tile has sub-128 fc1 chains (GELU turnaround can't keep up with those)
TILES = [(0, 512), (512, 512), (1024, 512), (1536, 40)]
NT = len(TILES)
NPTOK_PAD = ((NPTOK + 15) // 16) * 16  # xbar DMA-transpose needs 16-row mult
WARMUP_TRANSPOSES = 30

ATOM = {'vm': 0, 'im': 1, 'cm': 2, 'sc': 3, 'cc': 4}
TASK_PAIRS = [('vm', 'sc'), ('vm', 'cc'), ('im', 'sc'), ('im', 'cc'),
              ('cm', 'sc'), ('cm', 'cc')]
SRC = [[ATOM[l], ATOM[r]] for l, r in TASK_PAIRS]
DST = [[ATOM[r], ATOM[l]] for l, r in TASK_PAIRS]


# ---------------------------------------------------------------------------
# Walrus in this container accepts at most ONE sync-wait per instruction.
# Tile emits multi-wait instructions; split the extras onto preceding
# same-engine wait-nops (engines execute in order, semantics preserved).
# ---------------------------------------------------------------------------

def _patched_drain_and_barrier(self, tick_clock, wait_clock):
    nc = self.nc
    drain_inst = nc.sync.drain()
    wait_clock.add_sem_waits(
        drain_inst.ins, ScopedClock({None: tick_clock.global_clock}))
    si = drain_inst.ins.sync_info
    waits = list(si.on_wait) if si is not None and si.on_wait else []
    if len(waits) > 1:
        drain_inst.ins.sync_info = bass_rust.SyncInfo(
            on_wait=waits[:1], on_update=list(si.on_update or []))
        for w in waits[1:]:
            nop = nc.sync.nop(nofuse=True, hint="drain_wait_split")
            nop.ins.sync_info = bass_rust.SyncInfo(on_wait=[w], on_update=[])
    nc.all_engine_barrier()
    assert self.sems is not None
    popped = nc._tile_sem_poison_stack.pop()
    assert popped is self._sem_poison
    nc.clear_and_free_semaphores(list(self.sems.allocated().values()))
    nc.all_engine_barrier()


tile.TileContext._drain_and_barrier = _patched_drain_and_barrier


def legalize_sync_waits(nc):
    n_split = 0
    for f in nc.m.functions:
        for bb in f.blocks:
            insts = bb.instructions
            new_list = []
            for inst in insts:
                si = inst.sync_info
                waits = list(si.on_wait) if si is not None and si.on_wait else []
                if len(waits) > 1:
                    for w in waits[1:]:
                        eng = nc.engines[inst.engine]
                        nop = eng.nop(nofuse=True, hint="wait_split")
                        cur = nc.cur_bb.bb.instructions
                        assert cur and cur[-1] is nop.ins
                        cur.pop()
                        nop.ins.sync_info = bass_rust.SyncInfo(
                            on_wait=[w], on_update=[])
                        new_list.append(nop.ins)
                        n_split += 1
                    inst.sync_info = bass_rust.SyncInfo(
                        on_wait=waits[:1], on_update=list(si.on_update or []))
                new_list.append(inst)
            if len(new_list) != len(insts):
                insts[:] = new_list
    return n_split


# ---------------------------------------------------------------------------
# Kernel builder
# ---------------------------------------------------------------------------

def build_kernel(debug=False, repeat=1):
    nc = bass.Bass(num_devices=N_CORES)

    x_p = nc.declare_dram_parameter("x_p", [NPTOK_PAD, D], BF16,
                                    isOutput=False)
    xclsT = nc.declare_dram_parameter("xclsT", [KD, 128, NTOK_CLS], F32,
                                      isOutput=False)
    w1hp = nc.declare_dram_parameter("w1hp", [KH, 128, D], BF16, isOutput=False)
    w2tp = nc.declare_dram_parameter("w2tp", [KH, 128, D], BF16, isOutput=False)
    b1p = nc.declare_dram_parameter("b1p", [128, KH], F32, isOutput=False)
    b1rp = nc.declare_dram_parameter("b1rp", [1, H], BF16, isOutput=False)
    b2rp = nc.declare_dram_parameter("b2rp", [1, D], BF16, isOutput=False)
    b2bcp = nc.declare_dram_parameter("b2bcp", [128, D], F32, isOutput=False)
    b2pp = nc.declare_dram_parameter("b2pp", [128, KD], F32, isOutput=False)
    winp = nc.declare_dram_parameter("winp", [D, NA * HC], BF16, isOutput=False)
    binp = nc.declare_dram_parameter("binp", [128, NA * KC], F32, isOutput=False)
    woutp = nc.declare_dram_parameter("woutp", [KD, 128, NA * HC], BF16,
                                      isOutput=False)
    boutp = nc.declare_dram_parameter("boutp", [1, NA * D], BF16, isOutput=False)
    ghatp = nc.declare_dram_parameter("ghatp", [128, KD * 2 * NCLS], F32,
                                      isOutput=False)
    bbexp = nc.declare_dram_parameter("bbexp", [128, 6], F32, isOutput=False)
    y = nc.declare_dram_parameter("y", [TOK, D], F32, isOutput=True)

    cc_in = nc.dram_tensor("cc_in", [NTOK_CLS, D], BF16)
    cc_out = nc.dram_tensor("cc_out", [NTOK_CLS // N_CORES, D], BF16)

    with tile.TileContext(nc) as tc:
        with tc.tile_pool(name="persist", bufs=1) as pp, \
             tc.tile_pool(name="xT", bufs=2) as xT_p, \
             tc.tile_pool(name="ysb", bufs=2) as y_p, \
             tc.tile_pool(name="ps_t", bufs=2, space="PSUM") as ps_t, \
             tc.tile_pool(name="ps_1", bufs=2, space="PSUM") as ps_1, \
             tc.tile_pool(name="ps_2", bufs=2, space="PSUM") as ps_2:

            ident = pp.tile([128, 128], F32, tag="ident", name="ident")
            make_identity(nc, ident)
            ident16 = pp.tile([128, 128], BF16, tag="ident16", name="ident16")
            make_identity(nc, ident16)

            # ---- persistent SBUF tensors -------------------------------
            w1_sb = [pp.tile([128, 4 * D], BF16, tag=f"w1_{g}", name=f"w1_{g}")
                     for g in range(KH // 4)]
            w2_sb = [pp.tile([128, 4 * D], BF16, tag=f"w2_{g}", name=f"w2_{g}")
                     for g in range(KH // 4)]
            hT = [pp.tile([128, FT], BF16, tag=f"hT_{j}", name=f"hT_{j}")
                  for j in range(KH)]
            xcT32t = pp.tile([128, KD * NTOK_CLS], F32, tag="xcT32",
                             name="xcT32")
            xcT32 = [xcT32t[:, k * NTOK_CLS:(k + 1) * NTOK_CLS]
                     for k in range(KD)]
            xcT16t = pp.tile([128, KD * NTOK_CLS], BF16, tag="xcT16",
                             name="xcT16")
            xcT16 = [xcT16t[:, k * NTOK_CLS:(k + 1) * NTOK_CLS]
                     for k in range(KD)]
            win_sb = [pp.tile([128, NA * HC], BF16, tag=f"win_{k}",
                              name=f"win_{k}") for k in range(KD)]
            SH = [pp.tile([128, 2 * NTOK_CLS], BF16, tag=f"SH_{k}",
                          name=f"SH_{k}") for k in range(KC)]
            b1_sb = pp.tile([128, KH], F32, tag="b1", name="b1")
            b1r_sb = pp.tile([1, H], BF16, tag="b1r", name="b1r")
            b2r_sb = pp.tile([1, D], BF16, tag="b2r", name="b2r")
            b2bc_sb = pp.tile([128, D], F32, tag="b2bc", name="b2bc")
            b2p_sb = pp.tile([128, KD], F32, tag="b2p", name="b2p")
            bin_sb = pp.tile([128, NA * KC], F32, tag="bin", name="bin")
            bout_sb = pp.tile([1, NA * D], BF16, tag="bout", name="bout")
            ghat_sb = pp.tile([128, KD * 12], F32, tag="ghat", name="ghat")
            ones_sb = pp.tile([1, 128], BF16, tag="ones", name="ones")
            w0T_sb = pp.tile([1, NTOK_CLS], F32, tag="w0T", name="w0T")
            w1T_sb_g = pp.tile([1, NTOK_CLS], F32, tag="w1Tg", name="w1Tg")
            w0T16 = pp.tile([1, NTOK_CLS], BF16, tag="w0T16", name="w0T16")
            w1T16 = pp.tile([1, NTOK_CLS], BF16, tag="w1T16", name="w1T16")
            W0b = pp.tile([128, NTOK_CLS], BF16, tag="W0b", name="W0b")
            W1b = pp.tile([128, NTOK_CLS], BF16, tag="W1b", name="W1b")
            zrow_sb = pp.tile([1, 128], BF16, tag="zrow", name="zrow")
            onescol = pp.tile([128, 1], BF16, tag="onescol", name="onescol")
            nc.vector.memset(onescol[:, :], 1.0)
            nc.vector.memset(ones_sb[:, :], 1.0)
            nc.vector.memset(zrow_sb[:, :], 0.0)

            for _rep in range(repeat):
                _emit_iteration(
                    nc, tc, debug,
                    x_p, xclsT, w1hp, w2tp, b1p, b1rp, b2rp, b2bcp, b2pp,
                    winp,
                    binp, woutp, boutp, ghatp, bbexp, y, cc_in, cc_out,
                    pp, xT_p, y_p, ps_t, ps_1, ps_2,
                    ident, ident16, w1_sb, w2_sb, hT, xcT32t, xcT32,
                    xcT16t, xcT16, win_sb, SH, b1_sb, b1r_sb, b2r_sb,
                    b2bc_sb,
                    b2p_sb,
                    bin_sb, bout_sb,
                    ghat_sb, ones_sb, onescol, w0T_sb, w1T_sb_g, w0T16, w1T16,
                    W0b, W1b, zrow_sb, load_weights=(_rep == 0))

    legalize_sync_waits(nc)
    return nc


def _emit_iteration(nc, tc, debug,
                    x_p, xclsT, w1hp, w2tp, b1p, b1rp, b2rp, b2bcp, b2pp,
                    winp,
                    binp, woutp, boutp, ghatp, bbexp, y, cc_in, cc_out,
                    pp, xT_p, y_p, ps_t, ps_1, ps_2,
                    ident, ident16, w1_sb, w2_sb, hT, xcT32t, xcT32,
                    xcT16t, xcT16, win_sb, SH, b1_sb, b1r_sb, b2r_sb,
                    b2bc_sb,
                    b2p_sb,
                    bin_sb, bout_sb,
                    ghat_sb, ones_sb, onescol, w0T_sb, w1T_sb_g, w0T16, w1T16,
                    W0b, W1b, zrow_sb, load_weights):
    def emit_xT(t):
        t0tok, ftl = TILES[t]
        fdma = ((ftl + 15) // 16) * 16
        xTt = xT_p.tile([128, KD * FT], BF16, tag="xT", name=f"xT{t}")
        tv = xTt.rearrange("p (k f) -> p k f", k=KD)
        nc.sync.dma_start(out=tv[:, :, :fdma],
                          in_=x_p[t0tok:t0tok + fdma, :], transpose=True)
        return xTt, ftl

    # ---- DMA issue order: x tile0, fc1 weights, cls inputs, x tile1,
    # cls weights, fc2 weights, x tail, atom-out weights ----------------
    if load_weights:
        nc.sync.dma_start(out=w1_sb[0][:, 0:D], in_=w1hp[0, :, :])

    def emit_xT0_split():
        t0tok, ftl = TILES[0]
        xTt = xT_p.tile([128, KD * FT], BF16, tag="xT", name="xT0")
        tv = xTt.rearrange("p (k f) -> p k f", k=KD)
        nc.sync.dma_start(out=tv[:, 0:3, :],
                          in_=x_p[t0tok:t0tok + ftl, 0:384], transpose=True)
        nc.sync.dma_start(out=tv[:, 3:6, :],
                          in_=x_p[t0tok:t0tok + ftl, 384:768], transpose=True)
        return xTt, ftl

    xT0 = emit_xT0_split()
    if load_weights:
        # keep the PE p-state warm across the DMA-bound startup window:
        # scratch transposes (never read) so the first real chains start
        # at full clock instead of ramping through the slow p-states
        for _ in range(WARMUP_TRANSPOSES):
            pw_ = ps_t.tile([128, 128], BF16, tag="tp", name="warm")
            nc.tensor.transpose(pw_[:, :], ident16[:, :], ident16[:, :])
    if load_weights:
        gv0 = w1_sb[0].rearrange("p (j f) -> p j f", j=4)
        nc.sync.dma_start(out=gv0[:, 1:4, :],
                          in_=w1hp[1:4, :, :].rearrange("j p f -> p j f"))
        nc.sync.dma_start(out=b1_sb[:, :], in_=b1p[:, :])
        nc.sync.dma_start(out=b1r_sb[:, :], in_=b1rp[:, :])
        nc.sync.dma_start(out=b2r_sb[:, :], in_=b2rp[:, :])
        gv1 = w1_sb[1].rearrange("p (j f) -> p j f", j=4)
        nc.sync.dma_start(out=gv1[:, 0:2, :],
                          in_=w1hp[4:6, :, :].rearrange("j p f -> p j f"))
        nc.sync.dma_start(out=gv1[:, 2:4, :],
                          in_=w1hp[6:8, :, :].rearrange("j p f -> p j f"))
        for g in range(2, KH // 4):
            gv = w1_sb[g].rearrange("p (j f) -> p j f", j=4)
            nc.sync.dma_start(
                out=gv[:, :, :],
                in_=w1hp[4 * g:4 * g + 4, :, :].rearrange("j p f -> p j f"))
    if load_weights:
        nc.sync.dma_start(out=ghat_sb[:, :], in_=ghatp[:, :])
        nc.sync.dma_start(out=bin_sb[:, :], in_=binp[:, :])
        nc.sync.dma_start(out=b2p_sb[:, :], in_=b2pp[:, :])
        for g in range(3):
            gv = w2_sb[g].rearrange("p (j f) -> p j f", j=4)
            nc.sync.dma_start(
                out=gv[:, :, :],
                in_=w2tp[4 * g:4 * g + 4, :, :].rearrange("j p f -> p j f"))
    xcv = xcT32t.rearrange("p (k f) -> p k f", k=KD)
    nc.sync.dma_start(out=xcv[:, :, :],
                      in_=xclsT.rearrange("k p f -> p k f"))
    if load_weights:
        nc.sync.dma_start(out=b2bc_sb[:, :], in_=b2bcp[:, :])

    def w1sl(j, k):
        return w1_sb[j // 4][:, (j % 4) * D + k * 128:(j % 4) * D + k * 128 + 128]

    def w2sl(k, half):
        return w2_sb[k // 4][:, (k % 4) * D + half * 384:
                             (k % 4) * D + half * 384 + 384]

    def emit_fc1(t, xTt, ftl):
        for j in range(KH):
            ph = ps_1.tile([128, FT], F32, tag="f1", name=f"f1_{t}_{j}")
            for k in range(KD):
                nc.tensor.matmul(ph[:, :ftl],
                                 lhsT=w1sl(j, k),
                                 rhs=xTt[:, k * FT:k * FT + ftl],
                                 start=(k == 0), stop=(k == KD - 1))
            nc.scalar.activation(hT[j][:, :ftl], ph[:, :ftl], AF.Gelu,
                                 bias=b1_sb[:, j:j + 1])

    def emit_fc2(t, ftl):
        t0tok = TILES[t][0]
        nblk = (ftl + 127) // 128
        for bkl in range(nblk):
            m0 = bkl * 128
            ml = min(128, ftl - m0)
            last = (t == NT - 1 and bkl == nblk - 1)
            ysb = y_p.tile([128, D], F32, tag="ysb", name=f"y_{t}_{bkl}")
            for half in range(2):
                po = ps_2.tile([128, 384], F32, tag="f2",
                               name=f"f2_{t}_{bkl}_{half}")
                for k in range(KH):
                    nc.tensor.matmul(po[:ml, :],
                                     lhsT=hT[k][:, m0:m0 + ml],
                                     rhs=w2sl(k, half),
                                     start=(k == 0), stop=(k == KH - 1))
                cs = slice(half * 384, (half + 1) * 384)
                nc.vector.tensor_tensor(ysb[:ml, cs], po[:ml, :],
                                        b2bc_sb[:ml, cs], ALU.add)
                if last:
                    # final block: store each column-half as soon as it is
                    # ready so the last transfer overlaps the last chains
                    r = t0tok + m0
                    end = r + ml
                    while r < end:
                        b_i, p_i = divmod(r, NP)
                        nxt = min(end, (b_i + 1) * NP)
                        ys = b_i * T + NCLS + p_i
                        nc.scalar.dma_start(
                            out=y[ys:ys + (nxt - r), cs],
                            in_=ysb[r - (t0tok + m0):nxt - (t0tok + m0), cs])
                        r = nxt
            if last:
                continue
            g = t0tok + m0
            end = g + ml
            r = g
            while r < end:
                b_i, p_i = divmod(r, NP)
                nxt = min(end, (b_i + 1) * NP)
                ys = b_i * T + NCLS + p_i
                nc.scalar.dma_start(out=y[ys:ys + (nxt - r), :],
                                    in_=ysb[r - g:nxt - g, :])
                r = nxt

    def emit_fc1_tail(t, xTt, ftl, hTt2):
        # pack 12 j-outputs per PSUM bank; bias lands via rank-1 matmuls so
        # one GELU drains the whole bank (Act per-op overhead would gate PE)
        for grp in range(2):
            ph = ps_1.tile([128, FT], F32, tag="f1", name=f"f1t_{grp}")
            for jj in range(12):
                j = grp * 12 + jj
                nc.tensor.matmul(ph[:, jj * 40:jj * 40 + ftl],
                                 lhsT=b1r_sb[:, j * 128:(j + 1) * 128],
                                 rhs=ones_sb[:, :ftl],
                                 start=(jj == 0), stop=False)
                for k in range(KD):
                    nc.tensor.matmul(ph[:, jj * 40:jj * 40 + ftl],
                                     lhsT=w1sl(j, k),
                                     rhs=xTt[:, k * FT:k * FT + ftl],
                                     start=False,
                                     stop=(k == KD - 1))
            nc.scalar.activation(hTt2[grp][:, :12 * 40], ph[:, :12 * 40],
                                 AF.Gelu)

    def emit_fc2_tail(t, ftl, hTt2):
        t0tok = TILES[t][0]
        po = ps_2.tile([128, 384], F32, tag="f2", name="f2t")
        yt = y_p.tile([128, D], BF16, tag="yt16", name="ytail")
        y40 = y_p.tile([128, D], F32, tag="ysb", name="y40")
        for dp in range(KD):
            nc.tensor.matmul(po[:, dp * 40:dp * 40 + ftl],
                             lhsT=b2r_sb[:, dp * 128:(dp + 1) * 128],
                             rhs=ones_sb[:, :ftl],
                             start=(dp == 0), stop=False)
            for k in range(KH):
                nc.tensor.matmul(po[:, dp * 40:dp * 40 + ftl],
                                 lhsT=w2_sb[k // 4][:, (k % 4) * D + dp * 128:
                                                   (k % 4) * D + (dp + 1) * 128],
                                 rhs=hTt2[k // 12][:, (k % 12) * 40:
                                                   (k % 12) * 40 + ftl],
                                 start=False,
                                 stop=(k == KH - 1))
        nc.scalar.activation(yt[:, :KD * 40], po[:, :KD * 40], AF.Identity)
        b_i, p_i = divmod(t0tok, NP)
        ys = b_i * T + NCLS + p_i
        for dp in range(KD):
            ptt = ps_t.tile([128, 128], BF16, tag="tp", name="tp16")
            nc.tensor.transpose(ptt[:ftl, :], yt[:, dp * 40:dp * 40 + ftl],
                                ident16[:, :])
            nc.vector.tensor_copy(y40[:ftl, dp * 128:(dp + 1) * 128],
                                  ptt[:ftl, :])
            if dp == 2:
                nc.sync.dma_start(out=y[ys:ys + ftl, 0:384],
                                  in_=y40[:ftl, 0:384])
        nc.sync.dma_start(out=y[ys:ys + ftl, 384:768], in_=y40[:ftl, 384:768])

    # ================= tile 0: fc1 =====================================
    xTt, ftl = xT0
    emit_fc1(0, xTt, ftl)
    if load_weights:
        for g in range(3, KH // 4):
            gv = w2_sb[g].rearrange("p (j f) -> p j f", j=4)
            nc.sync.dma_start(
                out=gv[:, :, :],
                in_=w2tp[4 * g:4 * g + 4, :, :].rearrange("j p f -> p j f"))
    xT1 = emit_xT(1)

    # ---- cls gate: norms, logits, route weights -----------------------
    with tc.tile_pool(name="cls_tmp", bufs=2) as cls_tmp, \
         tc.tile_pool(name="ps_g", bufs=2, space="PSUM") as ps_g:
        pn = ps_g.tile([128, NTOK_CLS], F32, tag="g", name="pn")
        for k in range(KD):
            nc.vector.tensor_copy(xcT16[k][:, :], xcT32[k][:, :])
            sq = cls_tmp.tile([128, NTOK_CLS], BF16, tag="sq", name=f"sq{k}")
            nc.scalar.activation(sq[:, :], xcT32[k][:, :], AF.Square)
            nc.tensor.matmul(pn[:1, :], lhsT=onescol[:, :], rhs=sq[:, :],
                             start=(k == 0), stop=(k == KD - 1))
        bb_all = cls_tmp.tile([128, 2 * 3], F32, tag="bba", name="bba")
        nc.sync.dma_start(out=bb_all[:, :], in_=bbexp[:, :])
        nrow = cls_tmp.tile([1, NTOK_CLS], F32, tag="nrow", name="nrow")
        nc.scalar.activation(nrow[:1, :], pn[:1, :], AF.Sqrt)
        rnrow = cls_tmp.tile([1, NTOK_CLS], F32, tag="rnrow", name="rnrow")
        nc.vector.reciprocal(rnrow[:1, :], nrow[:1, :])
        for i in range(3):  # 3 tiles of 128 cls tokens, (t,b) order
            ptn_ = ps_g.tile([128, NTOK_CLS], F32, tag="g", name="ptn")
            nc.tensor.transpose(ptn_[:, 0:1], rnrow[:1, i * 128:(i + 1) * 128],
                                ident[:1, :1])
            rn = cls_tmp.tile([128, 1], F32, tag="rn", name="rn")
            nc.vector.tensor_copy(rn[:, :], ptn_[:, 0:1])

            pgt = ps_g.tile([128, NTOK_CLS], F32, tag="g", name="pg")
            pg = pgt[:, 0:12]
            for k in range(KD):
                nc.tensor.matmul(
                    pg[:, :],
                    lhsT=xcT32[k][:, i * 128:(i + 1) * 128],
                    rhs=ghat_sb[:, k * 12:(k + 1) * 12],
                    start=(k == 0), stop=(k == KD - 1))
            lg = cls_tmp.tile([128, 12], F32, tag="lg", name="lg")
            nc.vector.tensor_scalar_mul(lg[:, :], pg[:, :], rn[:, :])

            bb_sb = bb_all[:, 2 * i:2 * i + 2]
            d01 = cls_tmp.tile([128, 2], F32, tag="d01", name="d01")
            t0, t1 = 2 * i, 2 * i + 1
            nc.vector.tensor_tensor(d01[0:64, :], lg[0:64, 2 * t0:2 * t0 + 2],
                                    bb_sb[0:64, :], ALU.add)
            nc.vector.tensor_tensor(d01[64:128, :],
                                    lg[64:128, 2 * t1:2 * t1 + 2],
                                    bb_sb[64:128, :], ALU.add)
            diff = cls_tmp.tile([128, 1], F32, tag="diff", name="diff")
            nc.vector.tensor_tensor(diff[:, :], d01[:, 0:1], d01[:, 1:2],
                                    ALU.subtract)
            ad = cls_tmp.tile([128, 1], F32, tag="ad", name="ad")
            nc.scalar.activation(ad[:, :], diff[:, :], AF.Abs)
            pmax = cls_tmp.tile([128, 1], F32, tag="pmax", name="pmax")
            nc.scalar.activation(pmax[:, :], ad[:, :], AF.Sigmoid)
            m0 = cls_tmp.tile([128, 1], F32, tag="m0", name="m0")
            nc.vector.tensor_scalar(m0[:, :], diff[:, :], 0.0, None, ALU.is_ge)
            w0 = cls_tmp.tile([128, 1], F32, tag="w0", name="w0")
            nc.vector.tensor_tensor(w0[:, :], m0[:, :], pmax[:, :], ALU.mult)
            w1g = cls_tmp.tile([128, 1], F32, tag="w1g", name="w1g")
            nc.vector.tensor_tensor(w1g[:, :], pmax[:, :], w0[:, :],
                                    ALU.subtract)
            ptw_ = ps_g.tile([128, NTOK_CLS], F32, tag="g", name="tpf")
            ptw = ptw_[:, 0:128]
            nc.tensor.transpose(ptw[:1, :], w0[:, 0:1], ident[:, :])
            nc.vector.tensor_copy(w0T_sb[:, i * 128:(i + 1) * 128], ptw[:1, :])
            ptw2_ = ps_g.tile([128, NTOK_CLS], F32, tag="g", name="tpf")
            ptw2 = ptw2_[:, 0:128]
            nc.tensor.transpose(ptw2[:1, :], w1g[:, 0:1], ident[:, :])
            nc.vector.tensor_copy(w1T_sb_g[:, i * 128:(i + 1) * 128],
                                  ptw2[:1, :])

        nc.vector.tensor_copy(w0T16[:, :], w0T_sb[:, :])
        nc.vector.tensor_copy(w1T16[:, :], w1T_sb_g[:, :])

        # broadcast w0/w1 across partitions, in SH column order.
        pw = ps_g.tile([128, NTOK_CLS], F32, tag="g", name="pw")
        ev = w0T16.rearrange("p (t b) -> p t b", b=64)
        nc.tensor.matmul(pw[:, 0:192], lhsT=ones_sb[:, :],
                         rhs=ev[:, 0:6:2, :], start=True, stop=True)
        nc.tensor.matmul(pw[:, 192:384], lhsT=ones_sb[:, :],
                         rhs=ev[:, 1:6:2, :], start=True, stop=True)
        nc.vector.tensor_copy(W0b[:, :], pw[:, :])
        pw2 = ps_g.tile([128, NTOK_CLS], F32, tag="g", name="pw")
        nc.tensor.matmul(pw2[:, :], lhsT=ones_sb[:, :],
                         rhs=w1T16[:, :], start=True, stop=True)
        nc.vector.tensor_copy(W1b[:, :], pw2[:, :])

    # ================= tile 0 fc2, tile 1 ==============================
    emit_fc2(0, ftl)
    xTt, ftl = xT1
    emit_fc1(1, xTt, ftl)
    xT2 = emit_xT(2)
    for k in range(KD):
        nc.sync.dma_start(out=win_sb[k][:, :],
                          in_=winp[k * 128:(k + 1) * 128, :])
    xT3 = emit_xT(3)
    if load_weights:
        nc.sync.dma_start(out=bout_sb[:, :], in_=boutp[:, :])

    # ---- cls stage-1: hid in SH layout --------------------------------
    # SH col layout: [dst3: t0,t2,t4 (192)][dst4: t1,t3,t5 (192)]
    #                [dst0: t0,t1 (128)][dst1: t2,t3][dst2: t4,t5]
    with tc.tile_pool(name="ps_s1", bufs=2, space="PSUM") as ps_s1:
        xvt = xcT16t.rearrange("p (k t b) -> p k t b", k=KD, b=64)
        xv = [xvt[:, k] for k in range(KD)]
        for a in range(NA):
            # pack two m-chains per PSUM tile: doubles the GELU drain
            # deadline so the Act round-trip never gates the ring
            for mp in ((0, 1), (2,)):
                na = 128 if a <= 2 else 192
                ph = ps_s1.tile([128, 384], F32, tag="s1",
                                name=f"s1_{a}_{mp[0]}")
                for idx, m in enumerate(mp):
                    for k in range(KD):
                        if a <= 2:
                            rhs = xcT16[k][:, a * 128:(a + 1) * 128]
                        else:
                            rhs = xv[k][:, (a - 3):NCLS:2, :]
                        nc.tensor.matmul(
                            ph[:, idx * 192:idx * 192 + na],
                            lhsT=win_sb[k][:, a * HC + m * 128:
                                           a * HC + (m + 1) * 128],
                            rhs=rhs,
                            start=(idx == 0 and k == 0),
                            stop=(k == KD - 1))
                for idx, m in enumerate(mp):
                    o = idx * 192
                    bias = bin_sb[:, a * KC + m: a * KC + m + 1]
                    if a <= 2:
                        # route-0 hid: task 2a -> dst3, 2a+1 -> dst4 group
                        nc.scalar.activation(
                            SH[m][:, a * 64:(a + 1) * 64],
                            ph[:, o:o + 64], AF.Gelu, bias=bias)
                        nc.scalar.activation(
                            SH[m][:, 192 + a * 64:192 + (a + 1) * 64],
                            ph[:, o + 64:o + 128], AF.Gelu, bias=bias)
                    else:
                        # route-1 hid: task t at 384 + (t//2)*128 + (t%2)*64
                        off = 64 * (a - 3)
                        for g in range(3):
                            nc.scalar.activation(
                                SH[m][:, 384 + g * 128 + off:
                                      384 + g * 128 + off + 64],
                                ph[:, o + g * 64:o + (g + 1) * 64],
                                AF.Gelu, bias=bias)
        for m in range(KC):
            nc.vector.tensor_tensor(SH[m][:, 0:384], SH[m][:, 0:384],
                                    W0b[:, :], ALU.mult)
            nc.vector.tensor_tensor(SH[m][:, 384:768], SH[m][:, 384:768],
                                    W1b[:, :], ALU.mult)

    # atom-out weights overwrite the atom-in tiles (sequential lifetimes)
    for k in range(KD):
        nc.sync.dma_start(out=win_sb[k][:, :], in_=woutp[k, :, :])

    def wout_ap(j, dp):
        flat = j * D + dp * 128
        return win_sb[flat // (NA * HC)][:, flat % (NA * HC):
                                         flat % (NA * HC) + 128]

    # ================= tile 1 fc2, tile 2 ==============================
    emit_fc2(1, ftl)
    xTt, ftl = xT2
    emit_fc1(2, xTt, ftl)

    # ---- cls stage-2: partial outputs, transpose, RS ------------------
    with tc.tile_pool(name="ps_s2", bufs=2, space="PSUM") as ps_s2, \
         tc.tile_pool(name="fin", bufs=2) as fin:
        pt_sb = [fin.tile([128, NTOK_CLS], BF16, tag=f"pt_{dp}",
                          name=f"pt_{dp}") for dp in range(KD)]
        shr = [SH[k].rearrange("p (q b) -> p q b", b=64) for k in range(KC)]
        w0r = w0T16.rearrange("p (t b) -> p t b", b=64)
        for dp in range(KD):
            pout = ps_s2.tile([128, NTOK_CLS], F32, tag="po", name=f"po_{dp}")
            nc.tensor.matmul(pout[:, :], lhsT=zrow_sb[:, :],
                             rhs=W0b[:1, :], start=True, stop=False)
            po = pout.rearrange("p (t b) -> p t b", b=64)
            for ai, a in enumerate((3, 4)):
                out_ap = po[:, ai:NCLS:2, :]
                for k in range(KC):
                    nc.tensor.matmul(
                        out_ap,
                        lhsT=wout_ap(a * KC + k, dp),
                        rhs=shr[k][:, 3 * ai:3 * (ai + 1), :],
                        start=False, stop=False)
            for a in range(3):
                out_ap = pout[:, a * 128:(a + 1) * 128]
                for k in range(KC):
                    nc.tensor.matmul(
                        out_ap,
                        lhsT=wout_ap(a * KC + k, dp),
                        rhs=SH[k][:, 384 + a * 128:384 + (a + 1) * 128],
                        start=False, stop=False)
            for ai, a in enumerate((3, 4)):
                nc.tensor.matmul(
                    po[:, ai:NCLS:2, :],
                    lhsT=bout_sb[:, a * D + dp * 128:a * D + (dp + 1) * 128],
                    rhs=w0r[:, ai:NCLS:2, :],
                    start=False, stop=False)
            for a in range(3):
                nc.tensor.matmul(
                    pout[:, a * 128:(a + 1) * 128],
                    lhsT=bout_sb[:, a * D + dp * 128:a * D + (dp + 1) * 128],
                    rhs=w1T16[:, a * 128:(a + 1) * 128],
                    start=False, stop=(a == 2))
            nc.vector.tensor_copy(pt_sb[dp][:, :], pout[:, :])

        for i in range(3):  # token blocks of 128 (t,b order)
            o3 = fin.tile([128, D], BF16, tag="o3", name=f"o3{i}")
            for dp in range(KD):
                ptt = ps_t.tile([128, 128], BF16, tag="tp", name="tp16")
                nc.tensor.transpose(ptt[:, :],
                                    pt_sb[dp][:, i * 128:(i + 1) * 128],
                                    ident16[:, :])
                nc.vector.tensor_copy(o3[:, dp * 128:(dp + 1) * 128],
                                      ptt[:, :])
            for half in range(2):
                t = 2 * i + half
                cc_view = cc_in.rearrange("(b t) d -> b t d", t=NCLS)
                nc.scalar.dma_start(
                    out=cc_view[:, t, :],
                    in_=o3[half * 64:(half + 1) * 64, :])

        nc.gpsimd.collective_compute(
            "ReduceScatter", ALU.add,
            replica_groups=[list(range(N_CORES))],
            ins=[cc_in[:, :]], outs=[cc_out[:, :]])

    # ================= tile 2 fc2, tile 3 (tail) =======================
    emit_fc2(2, ftl)
    xTt, ftl = xT3
    hTt2 = [pp.tile([128, 12 * 40], BF16, tag=f"hTt_{g}", name=f"hTt_{g}")
            for g in range(2)]
    emit_fc1_tail(3, xTt, ftl, hTt2)
    emit_fc2_tail(3, ftl, hTt2)

    # ---- cls output: convert RS result to f32, store ------------------
    with tc.tile_pool(name="cout", bufs=1) as cout_p:
        cco16 = cout_p.tile([NTOK_CLS // N_CORES, D], BF16, tag="cco16",
                            name="cco16")
        nc.scalar.dma_start(out=cco16[:, :], in_=cc_out[:, :])
        cco32 = cout_p.tile([NTOK_CLS // N_CORES, D], F32, tag="cco32",
                            name="cco32")
        nc.vector.tensor_copy(cco32[:, :], cco16[:, :])
        y_view = y.rearrange("(b t) d -> b t d", t=T)
        nc.scalar.dma_start(out=y_view[:, 0:NCLS, :], in_=cco32[:, :])


# ---------------------------------------------------------------------------
# Host side
# ---------------------------------------------------------------------------

_CACHE = {}


def _prep_inputs(x, fc1_w, fc1_b, fc2_w, fc2_b, gate_pair, atom_in_w, atom_in_b,
                 atom_out_w, atom_out_b, balance_bias):
    bf = ml_dtypes.bfloat16
    x = np.asarray(x, np.float32)
    w1T = np.asarray(fc1_w, np.float32).T  # [D, H]
    common = {
        # fc1 weights, h-major tiles: w1hp[j][dk, k*128+hj] = W1T[k*128+dk, j*128+hj]
        "w1hp": np.ascontiguousarray(
            w1T.reshape(KD, 128, KH, 128).transpose(2, 1, 0, 3)
            .reshape(KH, 128, D)).astype(bf),
        # fc2 weights as rhs tiles: w2tp[k] = fc2_w.T[k*128:(k+1)*128, :]
        "w2tp": np.ascontiguousarray(
            np.asarray(fc2_w, np.float32).T.reshape(KH, 128, D)).astype(bf),
        "b1p": np.ascontiguousarray(
            np.asarray(fc1_b, np.float32).reshape(KH, 128).T),
        "b1rp": np.asarray(fc1_b, np.float32).reshape(1, H).astype(bf),
        "b2rp": np.asarray(fc2_b, np.float32).reshape(1, D).astype(bf),
        "b2bcp": np.ascontiguousarray(
            np.broadcast_to(np.asarray(fc2_b, np.float32)[None, :], (128, D))),
        "b2pp": np.ascontiguousarray(
            np.asarray(fc2_b, np.float32).reshape(KD, 128).T),
        "boutp": (np.asarray(atom_out_b, np.float32) / N_CORES)
            .reshape(1, NA * D).astype(bf),
        "bbexp": np.ascontiguousarray(
            np.repeat(np.asarray(balance_bias, np.float32), B, axis=0)
            .reshape(3, 128, 2).transpose(1, 0, 2).reshape(128, 6)),
    }
    g = np.asarray(gate_pair, np.float32)
    gn = g / np.clip(np.linalg.norm(g, axis=-1, keepdims=True), 1e-12, None)
    ghatT = gn.reshape(2 * NCLS, D).T  # [D, 12]
    common["ghatp"] = np.ascontiguousarray(
        ghatT.reshape(KD, 128, 2 * NCLS).transpose(1, 0, 2)
        .reshape(128, KD * 2 * NCLS))
    # cls tokens for all batches in (t, b) order
    xc = np.asarray(x[:, :NCLS, :], np.float32)  # [B, 6, D]
    common["xclsT"] = np.ascontiguousarray(
        xc.transpose(1, 0, 2).reshape(NTOK_CLS, D).T.reshape(KD, 128,
                                                             NTOK_CLS))

    aiw = np.asarray(atom_in_w, np.float32)   # [5, H, D]
    aib = np.asarray(atom_in_b, np.float32)   # [5, H]
    aow = np.asarray(atom_out_w, np.float32)  # [5, D, H]

    in_maps = []
    for c in range(N_CORES):
        hs = slice(c * HC, (c + 1) * HC)
        m = dict(common)
        xp = np.zeros((NPTOK_PAD, D), np.float32)
        xp[:NPTOK] = x[c * BC:(c + 1) * BC, NCLS:, :].reshape(NPTOK, D)
        m["x_p"] = xp.astype(bf)
        m["winp"] = np.ascontiguousarray(
            aiw[:, hs, :].transpose(2, 0, 1).reshape(D, NA * HC)).astype(bf)
        m["binp"] = np.ascontiguousarray(
            aib[:, hs].reshape(NA, KC, 128).transpose(2, 0, 1)
            .reshape(128, NA * KC))
        blocks = (aow[:, :, hs].transpose(0, 2, 1)
                  .reshape(NA, KC, 128, D).reshape(NA * KC, 128, D))
        flatcols = np.concatenate(list(blocks), axis=1)  # [128, 11520]
        m["woutp"] = np.ascontiguousarray(
            flatcols.reshape(128, KD, NA * HC).swapaxes(0, 1)
        ).astype(bf)
        in_maps.append(m)
    return in_maps


def _get_nc():
    if "nc" not in _CACHE:
        _CACHE["nc"] = build_kernel()
    return _CACHE["nc"]


def kernel(**inputs) -> np.ndarray:
    nc = _get_nc()
    in_maps = _prep_inputs(**inputs)
    res = run_bass_kernel_spmd(nc, in_maps, core_ids=list(range(N_CORES)))
    out = np.empty((B, T, D), np.float32)
    for c in range(N_CORES):
        out[c * BC:(c + 1) * BC] = res.results[c]["y"].reshape(BC, T, D)
    return out


if __name__ == "__main__":
    nc = build_kernel()
    n = sum(len(bb.instructions) for f in nc.m.functions for bb in f.blocks)
    print("instructions:", n)

